# revision 1
# baseline (speedup 1.0000x reference)
"""Multi-head attention (B=2, S=2048, E=2048, H=16, causal) on 8 TRN2 NeuronCores.

Sharding: 8 cores = 2 batch shards x 4 head-group shards (4 heads / 512
features each).  Each core runs the full attention stack for its (batch,
head-group) and produces a partial [S, E] output through its row-block of
Wo; the host sums the 4 partials per batch.

All matmuls run as float32r (full PE rate for free dim >= 256).
"""

import numpy as np

import concourse.bacc as bacc
import concourse.mybir as mybir
import concourse.tile as tile
from concourse import bass_utils

B, S, E, H = 2, 2048, 2048, 16
D = 128                    # head dim
HL = 4                     # heads per core
F = HL * D                 # local features = 512
EO = E // 128              # 16 contraction chunks
EG = 2                     # eo chunks per DMA group
TT = 256                   # phase-1 token tile
IT = 512                   # phase-2 query tile
F32 = mybir.dt.float32
F32R = mybir.dt.float32r
EXP = mybir.ActivationFunctionType.Exp
SCALE = 1.0 / float(np.sqrt(D))

_CACHE = {}


def _build():
    nc = bacc.Bacc("TRN2", target_bir_lowering=False, debug=False)
    xT = nc.dram_tensor("xT", [E, S], F32, kind="ExternalInput").ap()
    wqT = nc.dram_tensor("wqT", [E, F], F32, kind="ExternalInput").ap()
    wkT = nc.dram_tensor("wkT", [E, F], F32, kind="ExternalInput").ap()
    wvT = nc.dram_tensor("wvT", [E, F], F32, kind="ExternalInput").ap()
    woT = nc.dram_tensor("woT", [F, E], F32, kind="ExternalInput").ap()
    # causal mask pairs: [pair, 128, 2, IT]
    cmask = nc.dram_tensor("cmask", [2, 128, 2, IT], F32, kind="ExternalInput").ap()
    y = nc.dram_tensor("y", [S, E], F32, kind="ExternalOutput").ap()

    xT_t = xT.rearrange("(eo ei) t -> ei eo t", ei=128).bitcast(F32R)
    wqT_t = wqT.rearrange("(eo ei) f -> ei eo f", ei=128).bitcast(F32R)
    wkT_t = wkT.rearrange("(eo ei) f -> ei eo f", ei=128).bitcast(F32R)
    wvT_t = wvT.rearrange("(eo ei) f -> ei eo f", ei=128).bitcast(F32R)
    woT_t = woT.rearrange("(fc fi) e -> fi fc e", fi=128).bitcast(F32R)

    with tile.TileContext(nc) as tc:
        with tc.tile_pool(name="persist", bufs=1) as persist:
            qT = persist.tile([128, HL, S], F32R, tag="qT")
            kT = persist.tile([128, HL, S], F32R, tag="kT")
            vN = persist.tile([128, S // 128, F], F32R, tag="vN")
            maskT = persist.tile([128, 2, 2, IT], F32, tag="maskT")
            onesT_f = persist.tile([128, 1], F32, tag="onesT_f")
            onesT = persist.tile([128, 1], F32R, tag="onesT")

            nc.vector.memset(onesT_f[:], 1.0)
            nc.vector.tensor_copy(onesT[:], onesT_f[:])

            # ---------- phase 1: q/k/v projections (two f-half passes) ----
            with (
                tc.tile_pool(name="wres_q", bufs=2) as wpool_q,
                tc.tile_pool(name="wres_kv", bufs=1) as wpool_kv,
                tc.tile_pool(name="xstream", bufs=2) as xpool,
                tc.tile_pool(name="ps_qk", bufs=5, space="PSUM") as ps_qk,
                tc.tile_pool(name="ps_v", bufs=3, space="PSUM") as ps_v,
            ):
                for fp in range(2):
                    f0 = fp * 256
                    wq_res = wpool_q.tile([128, EO, 256], F32R, tag="wq")
                    wk_res = wpool_kv.tile([128, EO, 256], F32R, tag="wk")
                    wv_res = wpool_kv.tile([128, EO, 256], F32R, tag="wv")
                    xt0 = xpool.tile([128, EO, TT], F32R, tag="xt")
                    # issue in consumption order: wq/x first, then wk, wv
                    for g0 in range(0, EO, EG):
                        nc.sync.dma_start(
                            wq_res[:, g0:g0 + EG, :],
                            wqT_t[:, g0:g0 + EG, f0:f0 + 256],
                        )
                        nc.sync.dma_start(
                            xt0[:, g0:g0 + EG, :], xT_t[:, g0:g0 + EG, 0:TT]
                        )
                    for g0 in range(0, EO, EG):
                        nc.sync.dma_start(
                            wk_res[:, g0:g0 + EG, :],
                            wkT_t[:, g0:g0 + EG, f0:f0 + 256],
                        )
                    for g0 in range(0, EO, EG):
                        nc.sync.dma_start(
                            wv_res[:, g0:g0 + EG, :],
                            wvT_t[:, g0:g0 + EG, f0:f0 + 256],
                        )
                    for tt in range(S // TT):
                        t0 = tt * TT
                        if fp == 0 and tt == 4:
                            nc.sync.dma_start(
                                maskT[:], cmask.rearrange("q p m i -> p q m i")
                            )
                        if tt == 0:
                            xt = xt0
                        else:
                            xt = xpool.tile([128, EO, TT], F32R, tag="xt")
                            for g0 in range(0, EO, EG):
                                nc.sync.dma_start(
                                    xt[:, g0:g0 + EG, :],
                                    xT_t[:, g0:g0 + EG, t0:t0 + TT],
                                )
                        for wres, dst in ((wq_res, qT), (wk_res, kT)):
                            for fc in range(2):
                                ps = ps_qk.tile([128, TT], F32, tag="pqk")
                                for eo in range(EO):
                                    nc.tensor.matmul(
                                        ps[:],
                                        wres[:, eo, fc * 128:(fc + 1) * 128],
                                        xt[:, eo, :],
                                        start=(eo == 0),
                                        stop=(eo == EO - 1),
                                    )
                                nc.vector.tensor_copy(
                                    dst[:, fp * 2 + fc, t0:t0 + TT], ps[:]
                                )
                        for tc2 in range(TT // 128):
                            ps = ps_v.tile([128, 256], F32, tag="pv")
                            for eo in range(EO):
                                nc.tensor.matmul(
                                    ps[:],
                                    xt[:, eo, tc2 * 128:(tc2 + 1) * 128],
                                    wv_res[:, eo, :],
                                    start=(eo == 0),
                                    stop=(eo == EO - 1),
                                )
                            nc.vector.tensor_copy(
                                vN[:, (t0 // 128) + tc2, f0:f0 + 256], ps[:]
                            )

            # ---------- phase 2: attention per head ----------------------
            with tc.tile_pool(name="wo", bufs=1) as wo_pool:
                wo_res = wo_pool.tile([128, HL, E], F32R, tag="wo")
                outT = wo_pool.tile([128, HL, S], F32R, tag="outT")
                for g0 in range(0, HL, 2):
                    nc.sync.dma_start(
                        wo_res[:, g0:g0 + 2, :], woT_t[:, g0:g0 + 2, :]
                    )

                with (
                    tc.tile_pool(name="ph2", bufs=6) as epool,
                    tc.tile_pool(name="ph2t", bufs=4) as tpool,
                    tc.tile_pool(name="ph2b", bufs=2) as small,
                    tc.tile_pool(name="ps_s", bufs=4, space="PSUM") as ps_s,
                    tc.tile_pool(name="ps_o", bufs=2, space="PSUM") as ps_o,
                    tc.tile_pool(name="ps_r", bufs=1, space="PSUM") as ps_r,
                    tc.tile_pool(name="ps_yb", bufs=1, space="PSUM") as ps_yb,
                    tc.tile_pool(name="ystb", bufs=2) as ystb_pool,
                ):
                    ready_y = []
                    done_y = set()

                    def emit_y_group():
                        tcb, et = ready_y.pop(0)
                        done_y.add((tcb, et))
                        Yb = ps_yb.tile([128, 512], F32, tag="Yb")
                        for fc in range(HL):
                            nc.tensor.matmul(
                                Yb[:],
                                outT[:, fc, tcb * 128:(tcb + 1) * 128],
                                wo_res[:, fc, et * 512:(et + 1) * 512],
                                start=(fc == 0),
                                stop=(fc == HL - 1),
                            )
                        yb = ystb_pool.tile([128, 512], F32, tag="yb")
                        nc.vector.tensor_copy(yb[:], Yb[:])
                        nc.sync.dma_start(
                            y[tcb * 128:(tcb + 1) * 128,
                              et * 512:(et + 1) * 512],
                            yb[:],
                        )
                    for p in range(S // IT):
                        i0 = p * IT
                        for h in range(HL):
                            h0 = h * 128
                            njc = (i0 + IT) // 128
                            O = ps_o.tile([128, IT], F32, tag="O")
                            R = ps_r.tile([1, IT], F32, tag="R")

                            def emit_scores(jc):
                                q_off = jc - (i0 // 128)
                                # diag chunk q: columns i < 128*q are fully
                                # masked -- compute only the valid slice
                                # (clamped so the free dim stays >= 256 for
                                # the f32r fast path)
                                off = 0 if q_off < 0 else min(128 * q_off, 256)
                                Sps = ps_s.tile([128, IT], F32, tag="S")
                                nc.tensor.matmul(
                                    Sps[:, off:],
                                    kT[:, h, jc * 128:(jc + 1) * 128],
                                    qT[:, h, i0 + off:i0 + IT],
                                    start=True,
                                    stop=True,
                                )
                                Et = epool.tile([128, IT], F32R, tag="E")
                                if q_off < 0:
                                    nc.scalar.activation(
                                        Et[:], Sps[:], EXP, scale=SCALE
                                    )
                                else:
                                    Etmp = tpool.tile([128, IT], F32, tag="Etmp")
                                    nc.scalar.activation(
                                        Etmp[:, off:], Sps[:, off:], EXP,
                                        scale=SCALE,
                                    )
                                    nc.vector.tensor_mul(
                                        Et[:, off:], Etmp[:, off:],
                                        maskT[:, q_off // 2, q_off % 2, off:],
                                    )
                                return Et, off

                            def emit_av(jc, Et, off):
                                nc.tensor.matmul(
                                    O[:, off:],
                                    vN[:, jc, h0:h0 + 128],
                                    Et[:, off:],
                                    start=(jc == 0),
                                    stop=(jc == njc - 1),
                                )
                                nc.tensor.matmul(
                                    R[:, off:],
                                    onesT[:],
                                    Et[:, off:],
                                    start=(jc == 0),
                                    stop=(jc == njc - 1),
                                )

                            # scores/exp run 4 chunks ahead of attn@v/rowsum
                            pending = []
                            for jc in range(njc):
                                Et, off = emit_scores(jc)
                                pending.append((jc, Et, off))
                                if jc == 2 and ready_y:
                                    emit_y_group()
                                if len(pending) > 3:
                                    emit_av(*pending.pop(0))
                            for item in pending:
                                emit_av(*item)
                            rec = small.tile([1, IT], F32, tag="rec")
                            nc.vector.reciprocal(rec[:], R[:])
                            RB = small.tile([128, IT], F32, tag="RB")
                            nc.gpsimd.partition_broadcast(RB[:], rec[:])
                            if h == HL - 1 and p == S // IT - 1:
                                # free the last O/R banks early so phase-3's
                                # psum pool isn't gated on the recip chain
                                Ocp = small.tile([128, IT], F32, tag="Ocp")
                                nc.vector.tensor_copy(Ocp[:], O[:])
                                nc.vector.tensor_mul(
                                    outT[:, h, i0:i0 + IT], Ocp[:], RB[:]
                                )
                            else:
                                nc.vector.tensor_mul(
                                    outT[:, h, i0:i0 + IT], O[:], RB[:]
                                )
                            if h == HL - 1:
                                for tcb_r in range(4 * p, 4 * p + 4):
                                    for et_r in range(E // 512):
                                        ready_y.append((tcb_r, et_r))

                # ------ phase 3: output projection ------------------------
                with (
                    tc.tile_pool(name="yst3", bufs=6) as yst_pool,
                    tc.tile_pool(name="ps_y", bufs=8, space="PSUM") as ps_y,
                ):
                    for tcb in range(S // 128):
                        tb0 = tcb * 128
                        for et in range(E // 512):
                            if (tcb, et) in done_y:
                                continue
                            Y = ps_y.tile([128, 512], F32, tag="Y")
                            for fc in range(HL):
                                nc.tensor.matmul(
                                    Y[:],
                                    outT[:, fc, tb0:tb0 + 128],
                                    wo_res[:, fc, et * 512:(et + 1) * 512],
                                    start=(fc == 0),
                                    stop=(fc == HL - 1),
                                )
                            yst = yst_pool.tile([128, 512], F32, tag="yst")
                            if et % 2 == 1:
                                nc.vector.tensor_copy(yst[:], Y[:])
                            else:
                                nc.scalar.copy(yst[:], Y[:])
                            nc.sync.dma_start(
                                y[tb0:tb0 + 128,
                                  et * 512:(et + 1) * 512],
                                yst[:],
                            )
    nc.compile()
    return nc


def _get_nc():
    if "nc" not in _CACHE:
        _CACHE["nc"] = _build()
    return _CACHE["nc"]


def make_in_maps(x, Wq, Wk, Wv, Wo):
    x = np.asarray(x, np.float32)
    Wq = np.asarray(Wq, np.float32)
    Wk = np.asarray(Wk, np.float32)
    Wv = np.asarray(Wv, np.float32)
    Wo = np.asarray(Wo, np.float32)

    jj = np.arange(128, dtype=np.int64)[:, None]
    ii = np.arange(IT, dtype=np.int64)[None, :]
    cm = np.stack(
        [(128 * q + jj <= ii).astype(np.float32) for q in range(4)]
    )  # [4, 128, IT]
    cmask = np.ascontiguousarray(
        cm.reshape(2, 2, 128, IT).transpose(0, 2, 1, 3)
    )  # [pair, 128, 2, IT]

    xTs = [np.ascontiguousarray(x[b].T) for b in range(B)]
    in_maps = []
    for c in range(8):
        b, g = c // 4, c % 4
        fsl = slice(F * g, F * (g + 1))
        in_maps.append({
            "xT": xTs[b],
            "wqT": np.ascontiguousarray(Wq[fsl, :].T),
            "wkT": np.ascontiguousarray(Wk[fsl, :].T),
            "wvT": np.ascontiguousarray(Wv[fsl, :].T),
            "woT": np.ascontiguousarray(Wo[:, fsl].T),
            "cmask": cmask,
        })
    return in_maps


def combine_outputs(results):
    out = np.empty((B, S, E), np.float32)
    for b in range(B):
        acc = results[4 * b]["y"].astype(np.float32).copy()
        for g in range(1, 4):
            acc += results[4 * b + g]["y"]
        out[b] = acc
    return out


def kernel(x, Wq, Wk, Wv, Wo):
    import time as _time

    nc = _get_nc()
    in_maps = make_in_maps(x, Wq, Wk, Wv, Wo)
    last_exc = None
    for attempt in range(3):
        if attempt:
            # transient device wedge (e.g. NRT_EXEC_UNIT_UNRECOVERABLE) --
            # wait for recovery before retrying
            _time.sleep(30 * attempt)
        try:
            res = bass_utils.run_bass_kernel_spmd(
                nc, in_maps, core_ids=list(range(8))
            )
            return combine_outputs(res.results)
        except Exception as exc:
            last_exc = exc
    raise last_exc



# revision 3
# speedup vs baseline: 1.1257x; 1.1257x over previous
"""Multi-head attention (B=2, S=2048, E=2048, H=16, causal) on 8 TRN2 NeuronCores.

Sharding: 8 cores = 2 batch shards x 4 head-group shards (4 heads / 512
features each).  Each core runs the full attention stack for its (batch,
head-group) and produces a partial [S, E] output through its row-block of
Wo; the host sums the 4 partials per batch.

Projections (QKV, Wo) run as 3-term fp8e4m3 hi/lo split matmuls in
DoubleRow perf mode (A@B ~ A1B1 + A2B1 + A1B2, each term contracting
2x128 rows per pass at 0.5 cyc/row).  The hi/lo splits of x and the
weights are prepared on the host; weights are pre-scaled by 64 so their
values sit in fp8's normal range (the scale is undone on the way out).
Attention (scores, attn@v, rowsum) runs in fp16 at full PE rate.
"""

import numpy as np

import concourse.bacc as bacc
import concourse.mybir as mybir
import concourse.tile as tile
from concourse import bass_utils

B, S, E, H = 2, 2048, 2048, 16
D = 128                    # head dim
HL = 4                     # heads per core
F = HL * D                 # local features = 512
EO = E // 128              # 16 contraction chunks
TT = 256                   # phase-1 token tile
IT = 512                   # phase-2 query tile
F32 = mybir.dt.float32
F16 = mybir.dt.float16
F8 = mybir.dt.float8e4
DR = mybir.MatmulPerfMode.DoubleRow
EXP = mybir.ActivationFunctionType.Exp
COPY = mybir.ActivationFunctionType.Copy
WS = 64.0                  # host-side weight prescale
SCALE = 1.0 / float(np.sqrt(D)) / (WS * WS)
NPF8 = mybir.dt.np(F8)

_CACHE = {}


def _build():
    nc = bacc.Bacc("TRN2", target_bir_lowering=False, debug=False)
    x1T = nc.dram_tensor("x1T", [E, S], F8, kind="ExternalInput").ap()
    x2T = nc.dram_tensor("x2T", [E, S], F8, kind="ExternalInput").ap()
    wq1 = nc.dram_tensor("wq1", [E, F], F8, kind="ExternalInput").ap()
    wq2 = nc.dram_tensor("wq2", [E, F], F8, kind="ExternalInput").ap()
    wk1 = nc.dram_tensor("wk1", [E, F], F8, kind="ExternalInput").ap()
    wk2 = nc.dram_tensor("wk2", [E, F], F8, kind="ExternalInput").ap()
    wv1 = nc.dram_tensor("wv1", [E, F], F8, kind="ExternalInput").ap()
    wv2 = nc.dram_tensor("wv2", [E, F], F8, kind="ExternalInput").ap()
    wo1 = nc.dram_tensor("wo1", [F, E], F8, kind="ExternalInput").ap()
    wo2 = nc.dram_tensor("wo2", [F, E], F8, kind="ExternalInput").ap()
    # causal mask pairs: [pair, 128, 2, IT] (fp16 0/1)
    cmask = nc.dram_tensor("cmask", [2, 128, 2, IT], F16, kind="ExternalInput").ap()
    y = nc.dram_tensor("y", [S, E], F32, kind="ExternalOutput").ap()

    x1_t = x1T.rearrange("(eo ei) t -> ei eo t", ei=128)
    x2_t = x2T.rearrange("(eo ei) t -> ei eo t", ei=128)
    w_t = {
        n: a.rearrange("(eo ei) f -> ei eo f", ei=128)
        for n, a in (("wq1", wq1), ("wq2", wq2), ("wk1", wk1),
                     ("wk2", wk2), ("wv1", wv1), ("wv2", wv2))
    }
    wo1_t = wo1.rearrange("(fc fi) e -> fi fc e", fi=128)
    wo2_t = wo2.rearrange("(fc fi) e -> fi fc e", fi=128)

    with tile.TileContext(nc) as tc:
        with tc.tile_pool(name="persist", bufs=1) as persist:
            qT = persist.tile([128, HL, S], F16, tag="qT")
            kT = persist.tile([128, HL, S], F16, tag="kT")
            vN = persist.tile([128, S // 128, F], F16, tag="vN")
            out1 = persist.tile([128, HL, S], F8, tag="out1")
            out2 = persist.tile([128, HL, S], F8, tag="out2")
            maskT = persist.tile([128, 2, 2, IT], F16, tag="maskT")
            onesT_f = persist.tile([128, 1], F32, tag="onesT_f")
            onesT = persist.tile([128, 1], F16, tag="onesT")

            # rowsum weights = WS so that rec = 1/(WS * sum(exp)) folds the
            # v-side prescale away in the normalize multiply
            nc.vector.memset(onesT_f[:], WS)
            nc.vector.tensor_copy(onesT[:], onesT_f[:])

            # ---------- phase 1: q/k/v projections (single pass) ----------
            with (
                tc.tile_pool(name="wres", bufs=1) as wpool,
                tc.tile_pool(name="xstream", bufs=4) as xpool,
                tc.tile_pool(name="ps_qk", bufs=5, space="PSUM") as ps_qk,
                tc.tile_pool(name="ps_v", bufs=2, space="PSUM") as ps_v,
            ):
                wres = {}
                for n in ("wq1", "wq2", "wk1", "wk2", "wv1", "wv2"):
                    wres[n] = wpool.tile([128, EO, F], F8, tag=n, name=n)
                # issue weight loads in consumption order, 4-chunk pieces
                for n in ("wq1", "wq2", "wk1", "wk2", "wv1", "wv2"):
                    for g0 in range(0, EO, 4):
                        nc.sync.dma_start(
                            wres[n][:, g0:g0 + 4, :], w_t[n][:, g0:g0 + 4, :]
                        )
                for tt in range(S // TT):
                    t0 = tt * TT
                    x1 = xpool.tile([128, EO, TT], F8, tag="x1")
                    x2 = xpool.tile([128, EO, TT], F8, tag="x2")
                    for g0 in range(0, EO, 8):
                        nc.sync.dma_start(
                            x1[:, g0:g0 + 8, :], x1_t[:, g0:g0 + 8, t0:t0 + TT]
                        )
                    for g0 in range(0, EO, 8):
                        nc.sync.dma_start(
                            x2[:, g0:g0 + 8, :], x2_t[:, g0:g0 + 8, t0:t0 + TT]
                        )
                    if tt == 4:
                        nc.sync.dma_start(
                            maskT[:], cmask.rearrange("q p m i -> p q m i")
                        )
                    for wn, dst in (("wq", qT), ("wk", kT)):
                        w1, w2 = wres[wn + "1"], wres[wn + "2"]
                        for fc in range(HL):
                            fsl = slice(fc * 128, (fc + 1) * 128)
                            ps = ps_qk.tile([128, TT], F32, tag="pqk")
                            terms = (
                                [(w1, x1, g) for g in range(0, EO, 2)]
                                + [(w2, x1, g) for g in range(0, EO, 2)]
                                + [(w1, x2, g) for g in range(0, EO, 2)]
                            )
                            for i, (w, x, g) in enumerate(terms):
                                nc.tensor.matmul(
                                    ps[:],
                                    w[:, g:g + 2, fsl],
                                    x[:, g:g + 2, :],
                                    start=(i == 0),
                                    stop=(i == len(terms) - 1),
                                    perf_mode=DR,
                                )
                            nc.vector.tensor_copy(dst[:, fc, t0:t0 + TT], ps[:])
                    w1, w2 = wres["wv1"], wres["wv2"]
                    for tc2 in range(TT // 128):
                        tsl = slice(tc2 * 128, (tc2 + 1) * 128)
                        ps = ps_v.tile([128, F], F32, tag="pv")
                        terms = (
                            [(x1, w1, g) for g in range(0, EO, 2)]
                            + [(x2, w1, g) for g in range(0, EO, 2)]
                            + [(x1, w2, g) for g in range(0, EO, 2)]
                        )
                        for i, (x, w, g) in enumerate(terms):
                            nc.tensor.matmul(
                                ps[:],
                                x[:, g:g + 2, tsl],
                                w[:, g:g + 2, :],
                                start=(i == 0),
                                stop=(i == len(terms) - 1),
                                perf_mode=DR,
                            )
                        nc.vector.tensor_copy(
                            vN[:, (t0 // 128) + tc2, :], ps[:]
                        )

            # ---------- phase 2: attention per head ----------------------
            with tc.tile_pool(name="wo", bufs=1) as wo_pool:
                wo1_r = wo_pool.tile([128, HL, E], F8, tag="wo1")
                wo2_r = wo_pool.tile([128, HL, E], F8, tag="wo2")
                nc.sync.dma_start(wo1_r[:], wo1_t)
                nc.sync.dma_start(wo2_r[:], wo2_t)

                with (
                    tc.tile_pool(name="ph2", bufs=6) as epool,
                    tc.tile_pool(name="ph2t", bufs=4) as tpool,
                    tc.tile_pool(name="ph2b", bufs=2) as small,
                    tc.tile_pool(name="ph2f", bufs=2) as fpool,
                    tc.tile_pool(name="ps_s", bufs=4, space="PSUM") as ps_s,
                    tc.tile_pool(name="ps_o", bufs=2, space="PSUM") as ps_o,
                    tc.tile_pool(name="ps_r", bufs=1, space="PSUM") as ps_r,
                    tc.tile_pool(name="ps_yb", bufs=1, space="PSUM") as ps_yb,
                    tc.tile_pool(name="ystb", bufs=2) as ystb_pool,
                ):
                    ready_y = []
                    done_y = set()

                    def emit_y_group():
                        tcb, et = ready_y.pop(0)
                        done_y.add((tcb, et))
                        tsl = slice(tcb * 128, (tcb + 1) * 128)
                        esl = slice(et * 512, (et + 1) * 512)
                        Yb = ps_yb.tile([128, 512], F32, tag="Yb")
                        terms = []
                        for fp in range(HL // 2):
                            g = 2 * fp
                            terms += [(out1, wo1_r, g), (out2, wo1_r, g),
                                      (out1, wo2_r, g)]
                        for i, (o, w, g) in enumerate(terms):
                            nc.tensor.matmul(
                                Yb[:],
                                o[:, g:g + 2, tsl],
                                w[:, g:g + 2, esl],
                                start=(i == 0),
                                stop=(i == len(terms) - 1),
                                perf_mode=DR,
                            )
                        yb = ystb_pool.tile([128, 512], F32, tag="yb")
                        nc.vector.tensor_scalar_mul(yb[:], Yb[:], 1.0 / WS)
                        nc.sync.dma_start(y[tsl, esl], yb[:])

                    for p in range(S // IT):
                        i0 = p * IT
                        for h in range(HL):
                            h0 = h * 128
                            njc = (i0 + IT) // 128
                            O = ps_o.tile([128, IT], F32, tag="O")
                            R = ps_r.tile([1, IT], F32, tag="R")

                            def emit_scores(jc):
                                q_off = jc - (i0 // 128)
                                # diag chunk q: columns i < 128*q are fully
                                # masked -- compute only the valid slice
                                off = 0 if q_off < 0 else 128 * q_off
                                Sps = ps_s.tile([128, IT], F32, tag="S")
                                nc.tensor.matmul(
                                    Sps[:, off:],
                                    kT[:, h, jc * 128:(jc + 1) * 128],
                                    qT[:, h, i0 + off:i0 + IT],
                                    start=True,
                                    stop=True,
                                )
                                Et = epool.tile([128, IT], F16, tag="E")
                                if q_off < 0:
                                    nc.scalar.activation(
                                        Et[:], Sps[:], EXP, scale=SCALE
                                    )
                                else:
                                    Etmp = tpool.tile([128, IT], F16, tag="Etmp")
                                    nc.scalar.activation(
                                        Etmp[:, off:], Sps[:, off:], EXP,
                                        scale=SCALE,
                                    )
                                    nc.vector.tensor_mul(
                                        Et[:, off:], Etmp[:, off:],
                                        maskT[:, q_off // 2, q_off % 2, off:],
                                    )
                                return Et, off

                            def emit_av(jc, Et, off):
                                nc.tensor.matmul(
                                    O[:, off:],
                                    vN[:, jc, h0:h0 + 128],
                                    Et[:, off:],
                                    start=(jc == 0),
                                    stop=(jc == njc - 1),
                                )
                                nc.tensor.matmul(
                                    R[:, off:],
                                    onesT[:],
                                    Et[:, off:],
                                    start=(jc == 0),
                                    stop=(jc == njc - 1),
                                )

                            # scores/exp run 4 chunks ahead of attn@v/rowsum
                            pending = []
                            for jc in range(njc):
                                Et, off = emit_scores(jc)
                                pending.append((jc, Et, off))
                                if jc == 2 and ready_y:
                                    emit_y_group()
                                if len(pending) > 3:
                                    emit_av(*pending.pop(0))
                            for item in pending:
                                emit_av(*item)
                            rec = small.tile([1, IT], F32, tag="rec")
                            nc.vector.reciprocal(rec[:], R[:])
                            RB = small.tile([128, IT], F32, tag="RB")
                            nc.gpsimd.partition_broadcast(RB[:], rec[:])
                            Ocp = fpool.tile([128, IT], F32, tag="Ocp")
                            nc.vector.tensor_mul(Ocp[:], O[:], RB[:])
                            nc.scalar.copy(out1[:, h, i0:i0 + IT], Ocp[:])
                            nc.vector.tensor_sub(
                                out2[:, h, i0:i0 + IT], Ocp[:],
                                out1[:, h, i0:i0 + IT],
                            )
                            if h == HL - 1:
                                for tcb_r in range(4 * p, 4 * p + 4):
                                    for et_r in range(E // 512):
                                        ready_y.append((tcb_r, et_r))

                # ------ phase 3: output projection ------------------------
                with (
                    tc.tile_pool(name="yst3", bufs=6) as yst_pool,
                    tc.tile_pool(name="ps_y", bufs=8, space="PSUM") as ps_y,
                ):
                    for tcb in range(S // 128):
                        tsl = slice(tcb * 128, (tcb + 1) * 128)
                        for et in range(E // 512):
                            if (tcb, et) in done_y:
                                continue
                            esl = slice(et * 512, (et + 1) * 512)
                            Y = ps_y.tile([128, 512], F32, tag="Y")
                            terms = []
                            for fp in range(HL // 2):
                                g = 2 * fp
                                terms += [(out1, wo1_r, g), (out2, wo1_r, g),
                                          (out1, wo2_r, g)]
                            for i, (o, w, g) in enumerate(terms):
                                nc.tensor.matmul(
                                    Y[:],
                                    o[:, g:g + 2, tsl],
                                    w[:, g:g + 2, esl],
                                    start=(i == 0),
                                    stop=(i == len(terms) - 1),
                                    perf_mode=DR,
                                )
                            yst = yst_pool.tile([128, 512], F32, tag="yst")
                            if et % 2 == 1:
                                nc.vector.tensor_scalar_mul(
                                    yst[:], Y[:], 1.0 / WS
                                )
                            else:
                                nc.scalar.activation(
                                    yst[:], Y[:], COPY, scale=1.0 / WS
                                )
                            nc.sync.dma_start(y[tsl, esl], yst[:])
    nc.compile()
    return nc


def _get_nc():
    if "nc" not in _CACHE:
        _CACHE["nc"] = _build()
    return _CACHE["nc"]


def _split8(a):
    hi = a.astype(NPF8)
    lo = (a - hi.astype(np.float32)).astype(NPF8)
    return hi, lo


def make_in_maps(x, Wq, Wk, Wv, Wo):
    x = np.asarray(x, np.float32)
    Wq = np.asarray(Wq, np.float32)
    Wk = np.asarray(Wk, np.float32)
    Wv = np.asarray(Wv, np.float32)
    Wo = np.asarray(Wo, np.float32)

    jj = np.arange(128, dtype=np.int64)[:, None]
    ii = np.arange(IT, dtype=np.int64)[None, :]
    cm = np.stack(
        [(128 * q + jj <= ii).astype(np.float16) for q in range(4)]
    )  # [4, 128, IT]
    cmask = np.ascontiguousarray(
        cm.reshape(2, 2, 128, IT).transpose(0, 2, 1, 3)
    )  # [pair, 128, 2, IT]

    xs = [_split8(np.ascontiguousarray(x[b].T)) for b in range(B)]
    in_maps = []
    for c in range(8):
        b, g = c // 4, c % 4
        fsl = slice(F * g, F * (g + 1))
        m = {"x1T": xs[b][0], "x2T": xs[b][1], "cmask": cmask}
        for n, W in (("wq", Wq), ("wk", Wk), ("wv", Wv)):
            hi, lo = _split8(np.ascontiguousarray(W[fsl, :].T) * WS)
            m[n + "1"], m[n + "2"] = hi, lo
        hi, lo = _split8(np.ascontiguousarray(Wo[:, fsl].T) * WS)
        m["wo1"], m["wo2"] = hi, lo
        in_maps.append(m)
    return in_maps


def combine_outputs(results):
    out = np.empty((B, S, E), np.float32)
    for b in range(B):
        acc = results[4 * b]["y"].astype(np.float32).copy()
        for g in range(1, 4):
            acc += results[4 * b + g]["y"]
        out[b] = acc
    return out


def kernel(x, Wq, Wk, Wv, Wo):
    import time as _time

    nc = _get_nc()
    in_maps = make_in_maps(x, Wq, Wk, Wv, Wo)
    last_exc = None
    for attempt in range(3):
        if attempt:
            # transient device wedge (e.g. NRT_EXEC_UNIT_UNRECOVERABLE) --
            # wait for recovery before retrying
            _time.sleep(30 * attempt)
        try:
            res = bass_utils.run_bass_kernel_spmd(
                nc, in_maps, core_ids=list(range(8))
            )
            return combine_outputs(res.results)
        except Exception as exc:
            last_exc = exc
    raise last_exc
